# revision 19
# baseline (speedup 1.0000x reference)
"""Trainium2 Bass kernel for the C4WithSyscalls soft-VM (moe_routing).

Key insight: with SCALE=10 the attention score gap between the matching
context key and any other key is 2*SCALE^2/sqrt(KDIM) = 44.7, so softmax
weights are exactly one-hot in fp32 (off weights ~e^-44.7 = 4e-20 vanish
under fp32 addition).  The soft attend/scatter is therefore bit-equivalent
to a hard gather/scatter.  Each VM step reduces to:

  inst = ctx[pc]; a = ctx[sp]             (per-row dynamic gather)
  gates/experts                            (elementwise, 15 experts)
  mem[sp] += pop_g*(new_ax - mem[sp])      (per-row dynamic scatter, sp<MEM)

Layout: 1024 rows data-parallel over 8 cores -> 128 rows/core, one row per
SBUF partition.  Gathers are indirect DMAs from the *immutable* input
tensors (memory + a small static tail for bp/out_len/output); values are
then patched from (1) a 16-entry in-SBUF scatter log via a last-wins
tensor_tensor_scan, and (2) the live pc/sp/ax registers.  Scatters are
indirect DMAs straight into the output tensor, behind a chunked base copy
of the input memory; the gather stream never depends on them, so the loop's
critical path is gather -> gates -> new_sp -> next gather.

All transcendentals use one activation table set (silu_and_others: silu +
tanh); sigmoid(x) = 0.5 + 0.5*tanh(x/2) and exp2(sh) = 2^floor(sh) *
(1+t)/(1-t) with t = tanh(frac*ln2/2), with 2^i built by integer shifts.
"""

import numpy as np

P = 128
MEM = 16384
OUTW = 64
NCTX = MEM + 5 + OUTW          # 16453
TAILW = 128                    # tail dram tensor width; cols 0..68 meaningful
BIGF = float(1 << 26)          # pushes an offset past every bounds check
MAGIC = 8388608.0              # 2^23, round-to-int magic constant
N_CORES = 8
OP_VALS = [1., 2., 9., 10., 11., 12., 13., 14., 15., 3., 4., 5., 6., 7., 8.]
LN2 = 0.6931471805599453

_BUILD_CACHE = {}


def build_bass(num_steps: int):
    import concourse.bass as bass
    import concourse.bacc as bacc
    import concourse.mybir as mybir
    import concourse.tile as tile

    f32 = mybir.dt.float32
    i32 = mybir.dt.int32
    AF = mybir.ActivationFunctionType
    ALU = mybir.AluOpType
    AXL = mybir.AxisListType
    IOff = bass.IndirectOffsetOnAxis

    # Bacc (not raw Bass): its compile pipeline splits multi-sem sync waits
    # into event semaphores, which direct 2D DMAs require (max 1 HW wait).
    nc = bacc.Bacc(num_swdge_queues=4)

    mem_in = nc.dram_tensor("memory", [P, MEM], f32, kind="ExternalInput")
    outp_in = nc.dram_tensor("output", [P, OUTW], f32, kind="ExternalInput")
    ax_in = nc.dram_tensor("ax", [P, 1], f32, kind="ExternalInput")
    pc_in = nc.dram_tensor("pc", [P, 1], i32, kind="ExternalInput")
    sp_in = nc.dram_tensor("sp", [P, 1], i32, kind="ExternalInput")
    bp_in = nc.dram_tensor("bp", [P, 1], i32, kind="ExternalInput")
    olf_in = nc.dram_tensor("out_len", [P, 1], i32, kind="ExternalInput")
    out_t = nc.dram_tensor("out", [P, MEM], f32, kind="ExternalOutput")
    tail_t = nc.dram_tensor("tail", [P, TAILW], f32, kind="Internal")

    mem_flat = mem_in[:, :].rearrange("p n -> (p n)")[:, None]
    out_flat = out_t[:, :].rearrange("p n -> (p n)")[:, None]
    tail_flat = tail_t[:, :].rearrange("p n -> (p n)")[:, None]

    with tile.TileContext(nc) as tc:
        with tc.tile_pool(name="state", bufs=1) as st, \
             tc.tile_pool(name="work", bufs=3) as wk:

            # ---------------- constants / init ----------------
            opv20 = st.tile([P, 15], f32)     # 20*op_val per column
            for j, vv in enumerate(OP_VALS):
                nc.vector.memset(opv20[:, j:j + 1], 20.0 * vv)
            opvn20 = st.tile([P, 15], f32)    # -20*op_val
            nc.vector.tensor_scalar(opvn20[:, :], opv20[:, :], -1.0, None, ALU.mult)

            posmem_i = st.tile([P, 1], i32)
            nc.gpsimd.iota(posmem_i[:, :], pattern=[[0, 1]], base=0,
                           channel_multiplier=MEM)
            posmem = st.tile([P, 1], f32)
            nc.vector.tensor_copy(posmem[:, :], posmem_i[:, :])
            postail_i = st.tile([P, 1], i32)
            nc.gpsimd.iota(postail_i[:, :], pattern=[[0, 1]], base=0,
                           channel_multiplier=TAILW)
            postail = st.tile([P, 1], f32)
            nc.vector.tensor_copy(postail[:, :], postail_i[:, :])

            log_idx = st.tile([P, 16], f32)
            nc.vector.memset(log_idx[:, :], -1.0)
            log_val = st.tile([P, 16], f32)
            nc.vector.memset(log_val[:, :], 0.0)

            # registers
            pci = st.tile([P, 1], i32)
            nc.sync.dma_start(pci[:, :], pc_in[:, :])
            spi = st.tile([P, 1], i32)
            nc.sync.dma_start(spi[:, :], sp_in[:, :])
            bpi = st.tile([P, 1], i32)
            nc.sync.dma_start(bpi[:, :], bp_in[:, :])
            olfi = st.tile([P, 1], i32)
            nc.sync.dma_start(olfi[:, :], olf_in[:, :])
            axf = st.tile([P, 1], f32)
            nc.sync.dma_start(axf[:, :], ax_in[:, :])

            pcf = st.tile([P, 1], f32)
            nc.vector.tensor_copy(pcf[:, :], pci[:, :])
            spf = st.tile([P, 1], f32)
            nc.vector.tensor_copy(spf[:, :], spi[:, :])
            bpf = st.tile([P, 1], f32)
            nc.vector.tensor_copy(bpf[:, :], bpi[:, :])
            olff = st.tile([P, 1], f32)
            nc.vector.tensor_copy(olff[:, :], olfi[:, :])

            # static tail: ctx index MEM+j -> tail col j.
            # cols 0(pc),1(sp),3(ax) stay 0 (patched from live regs);
            # col2=bp, col4=out_len, cols 5..68=output.
            tail_s = st.tile([P, TAILW], f32)
            nc.vector.memset(tail_s[:, :], 0.0)
            nc.vector.tensor_copy(tail_s[:, 2:3], bpf[:, :])
            nc.vector.tensor_copy(tail_s[:, 4:5], olff[:, :])
            outp_s = st.tile([P, OUTW], f32)
            nc.sync.dma_start(outp_s[:, :], outp_in[:, :])
            nc.vector.tensor_copy(tail_s[:, 5:5 + OUTW], outp_s[:, :])
            nc.sync.dma_start(tail_t[:, :], tail_s[:, :])

            # base copy input memory -> output, bounced through SBUF
            # (direct DRAM->DRAM pseudo-DMAs can't carry sync waits)
            NCH = 8
            CW = MEM // NCH
            for c in range(NCH):
                chunk = st.tile([P, CW], f32, tag=f"basecopy{c}")
                nc.sync.dma_start(chunk[:, :], mem_in[:, c * CW:(c + 1) * CW])
                # store via SWDGE: HWDGE direct2d DMAs only support a single
                # embedded sync wait, which the scheduler can exceed here
                nc.gpsimd.dma_start(out_t[:, c * CW:(c + 1) * CW], chunk[:, :])

            def ts(out, in0, s1, s2, op0, op1=None):
                if op1 is None:
                    nc.vector.tensor_scalar(out, in0, s1, None, op0)
                else:
                    nc.vector.tensor_scalar(out, in0, s1, s2, op0, op1)

            def tt(out, a, b, op):
                nc.vector.tensor_tensor(out=out, in0=a, in1=b, op=op)

            def stt(out, in0, scalar, in1, op0, op1):
                nc.vector.scalar_tensor_tensor(out, in0, scalar, in1, op0, op1)

            def floor_small(x_ap, tag):
                """floor(x) for |x| < 2^22."""
                r = wk.tile([P, 1], f32, tag=f"fs_r_{tag}")
                ts(r[:, :], x_ap, MAGIC, MAGIC, ALU.add, ALU.subtract)
                g = wk.tile([P, 1], f32, tag=f"fs_g_{tag}")
                tt(g[:, :], r[:, :], x_ap, ALU.is_gt)
                f = wk.tile([P, 1], f32, tag=f"fs_f_{tag}")
                tt(f[:, :], r[:, :], g[:, :], ALU.subtract)
                return f

            def floor_big(x_ap, tag):
                """floor(x) for any finite x (|x|>=2^23 is already integral)."""
                f = floor_small(x_ap, tag)
                ab = wk.tile([P, 1], f32, tag=f"fb_ab_{tag}")
                stt(ab[:, :], x_ap, -1.0, x_ap, ALU.mult, ALU.max)
                bigm = wk.tile([P, 1], f32, tag=f"fb_m_{tag}")
                ts(bigm[:, :], ab[:, :], MAGIC, None, ALU.is_ge)
                d = wk.tile([P, 1], f32, tag=f"fb_d_{tag}")
                tt(d[:, :], x_ap, f[:, :], ALU.subtract)
                m = wk.tile([P, 1], f32, tag=f"fb_mm_{tag}")
                tt(m[:, :], bigm[:, :], d[:, :], ALU.mult)
                fo = wk.tile([P, 1], f32, tag=f"fb_fo_{tag}")
                tt(fo[:, :], f[:, :], m[:, :], ALU.add)
                return fo

            # ---------------- the VM steps ----------------
            import os as _os
            STAGE = int(_os.environ.get("KSTAGE", "99"))
            for k in range(num_steps):
                # ---- A: gather indices + offsets (all f32, exact ints) ----
                idxf = wk.tile([P, 2], f32)
                ts(idxf[:, 0:1], pcf[:, :], float(NCTX - 1), None, ALU.min)
                spc = wk.tile([P, 1], f32)
                ts(spc[:, :], spf[:, :], 0.0, float(NCTX - 1), ALU.max, ALU.min)
                spfl = floor_small(spc[:, :], "sp")
                nc.vector.tensor_copy(idxf[:, 1:2], spfl[:, :])

                gem = wk.tile([P, 2], f32)   # 1.0 where idx >= MEM
                ts(gem[:, :], idxf[:, :], float(MEM), None, ALU.is_ge)
                ltm = wk.tile([P, 2], f32)   # 1.0 where idx < MEM
                ts(ltm[:, :], idxf[:, :], float(MEM), None, ALU.is_lt)

                offm_f = wk.tile([P, 2], f32)
                ts(offm_f[:, :], idxf[:, :], posmem[:, 0:1], None, ALU.add)
                stt(offm_f[:, :], gem[:, :], BIGF, offm_f[:, :], ALU.mult, ALU.add)
                offm = wk.tile([P, 2], i32)
                nc.vector.tensor_copy(offm[:, :], offm_f[:, :])

                offt_f = wk.tile([P, 2], f32)
                ts(offt_f[:, :], idxf[:, :], postail[:, 0:1], -float(MEM),
                   ALU.add, ALU.add)
                stt(offt_f[:, :], ltm[:, :], BIGF, offt_f[:, :], ALU.mult, ALU.add)
                offt = wk.tile([P, 2], i32)
                nc.vector.tensor_copy(offt[:, :], offt_f[:, :])

                if STAGE < 2:
                    continue
                # ---- B: the two gathers (memory part + static tail part) ----
                gm = wk.tile([P, 2], f32)
                nc.vector.memset(gm[:, :], 0.0)
                nc.gpsimd.indirect_dma_start(
                    out=gm[:, :], out_offset=None, in_=mem_flat,
                    in_offset=IOff(ap=offm[:, :], axis=0),
                    bounds_check=P * MEM - 1, oob_is_err=False)
                gt2 = wk.tile([P, 2], f32)
                nc.vector.memset(gt2[:, :], 0.0)
                nc.gpsimd.indirect_dma_start(
                    out=gt2[:, :], out_offset=None, in_=tail_flat,
                    in_offset=IOff(ap=offt[:, :], axis=0),
                    bounds_check=P * TAILW - 1, oob_is_err=False)
                v = wk.tile([P, 2], f32)
                tt(v[:, :], gm[:, :], gt2[:, :], ALU.add)
                if STAGE < 3:
                    continue

                # ---- C1: patch stale memory reads from the scatter log ----
                if k > 0:
                    for l in (0, 1):
                        eq = wk.tile([P, 16], f32, tag=f"leq{l}")
                        ts(eq[:, :k], log_idx[:, :k], idxf[:, l:l + 1], None,
                           ALU.is_equal)
                        om = wk.tile([P, 16], f32, tag=f"lom{l}")
                        ts(om[:, :k], eq[:, :k], -1.0, 1.0, ALU.mult, ALU.add)
                        ev = wk.tile([P, 16], f32, tag=f"lev{l}")
                        tt(ev[:, :k], eq[:, :k], log_val[:, :k], ALU.mult)
                        sc = wk.tile([P, 16], f32, tag=f"lsc{l}")
                        nc.vector.tensor_tensor_scan(
                            sc[:, :k], om[:, :k], ev[:, :k],
                            initial=v[:, l:l + 1], op0=ALU.mult, op1=ALU.add)
                        nc.vector.tensor_copy(v[:, l:l + 1], sc[:, k - 1:k])

                # ---- C2: patch live registers (ctx cols MEM,MEM+1,MEM+3) ----
                for cidx, reg in ((MEM, pcf), (MEM + 1, spf), (MEM + 3, axf)):
                    eqr = wk.tile([P, 2], f32, tag=f"eqr{cidx - MEM}")
                    ts(eqr[:, :], idxf[:, :], float(cidx), None, ALU.is_equal)
                    mm = wk.tile([P, 2], f32, tag=f"mm{cidx - MEM}")
                    tt(mm[:, :], eqr[:, :], v[:, :], ALU.mult)
                    v2 = wk.tile([P, 2], f32, tag=f"v2{cidx - MEM}")
                    tt(v2[:, :], v[:, :], mm[:, :], ALU.subtract)
                    er = wk.tile([P, 2], f32, tag=f"er{cidx - MEM}")
                    ts(er[:, :], eqr[:, :], reg[:, 0:1], None, ALU.mult)
                    tt(v[:, :], er[:, :], v2[:, :], ALU.add)

                inst = v[:, 0:1]
                a = v[:, 1:2]

                if STAGE < 31:
                    continue
                # ---- D: decode imm/opcode ----
                tq = wk.tile([P, 1], f32)
                ts(tq[:, :], inst, 1.0 / 256.0, None, ALU.mult)
                imm = floor_big(tq[:, :], "imm")
                opcode = wk.tile([P, 1], f32)
                stt(opcode[:, :], imm[:, :], -256.0, inst, ALU.mult, ALU.add)

                if STAGE < 32:
                    continue
                # ---- E: gates via one sigmoid batch [P, 67] ----
                # silu(x) = x*sigmoid(x), matching jax.nn.silu's decomposition.
                # cols 0-59: gate u-args; 60-63: eq2 args; 64: lt; 65: gt;
                # 66: exp2 fraction (e^y = s/(1-s), y = frac*ln2).
                c1 = wk.tile([P, 1], f32)
                ts(c1[:, :], opcode[:, :], 20.0, None, ALU.mult)
                d2 = wk.tile([P, 1], f32)
                tt(d2[:, :], a, axf[:, :], ALU.subtract)  # a - b
                c2 = wk.tile([P, 1], f32)
                ts(c2[:, :], d2[:, :], 20.0, None, ALU.mult)

                sh = wk.tile([P, 1], f32)
                ts(sh[:, :], axf[:, :], 31.0, 0.0, ALU.min, ALU.max)
                shfl = floor_small(sh[:, :], "sh")
                frac = wk.tile([P, 1], f32)
                tt(frac[:, :], sh[:, :], shfl[:, :], ALU.subtract)

                A = wk.tile([P, 67], f32)
                # u = 20*opcode - 20*v  per expert
                ts(A[:, 0:15], opvn20[:, :], c1[:, 0:1], 20.0, ALU.add, ALU.add)
                ts(A[:, 15:30], opvn20[:, :], c1[:, 0:1], None, ALU.add)
                ts(A[:, 30:45], opv20[:, :], c1[:, 0:1], 20.0, ALU.subtract,
                   ALU.add)
                ts(A[:, 45:60], opv20[:, :], c1[:, 0:1], None, ALU.subtract)
                ts(A[:, 60:61], c2[:, :], 20.0, None, ALU.add)
                nc.vector.tensor_copy(A[:, 61:62], c2[:, :])
                ts(A[:, 62:63], c2[:, :], -1.0, 20.0, ALU.mult, ALU.add)
                ts(A[:, 63:64], c2[:, :], -1.0, None, ALU.mult)
                ts(A[:, 64:65], d2[:, :], -20.0, -10.0, ALU.mult, ALU.add)
                ts(A[:, 65:66], d2[:, :], 20.0, -10.0, ALU.mult, ALU.add)
                ts(A[:, 66:67], frac[:, :], LN2, None, ALU.mult)

                if STAGE < 33:
                    continue
                S = wk.tile([P, 67], f32)
                nc.scalar.activation(S[:, :], A[:, :], AF.Sigmoid)
                SIL = wk.tile([P, 64], f32)
                tt(SIL[:, :], A[:, 0:64], S[:, 0:64], ALU.mult)

                st1 = wk.tile([P, 15], f32)
                tt(st1[:, :], SIL[:, 0:15], SIL[:, 15:30], ALU.subtract)
                st2 = wk.tile([P, 15], f32)
                tt(st2[:, :], SIL[:, 30:45], SIL[:, 45:60], ALU.subtract)
                gates = wk.tile([P, 15], f32)
                stt(gates[:, :], st1[:, :], 1.0 / 400.0, st2[:, :],
                    ALU.mult, ALU.mult)

                e1 = wk.tile([P, 1], f32)
                tt(e1[:, :], SIL[:, 60:61], SIL[:, 61:62], ALU.subtract)
                e2t = wk.tile([P, 1], f32)
                tt(e2t[:, :], SIL[:, 62:63], SIL[:, 63:64], ALU.subtract)
                eq2 = wk.tile([P, 1], f32)
                stt(eq2[:, :], e1[:, :], 1.0 / 400.0, e2t[:, :],
                    ALU.mult, ALU.mult)

                if STAGE < 34:
                    continue
                gsum = wk.tile([P, 1], f32)
                nc.vector.tensor_reduce(gsum[:, :], gates[:, :], axis=AXL.X,
                                        op=ALU.add)
                popg = wk.tile([P, 1], f32)
                nc.vector.tensor_reduce(popg[:, :], gates[:, 2:15], axis=AXL.X,
                                        op=ALU.add)

                if STAGE < 35:
                    continue
                # ---- F: exp2 from the sigmoid batch ----
                # 2^floor(sh) via exponent-field construction:
                # (floor(sh)+127)*2^23 is exact in fp32; cast to i32, bitcast.
                shexp = wk.tile([P, 1], f32)
                ts(shexp[:, :], shfl[:, :], 127.0, MAGIC, ALU.add, ALU.mult)
                e2i = wk.tile([P, 1], i32)
                nc.vector.tensor_copy(e2i[:, :], shexp[:, :])
                e2i_f = e2i[:, :].bitcast(f32)
                den = wk.tile([P, 1], f32)
                ts(den[:, :], S[:, 66:67], -1.0, 1.0, ALU.mult, ALU.add)
                rden = wk.tile([P, 1], f32)
                nc.vector.reciprocal(rden[:, :], den[:, :])
                e2f = wk.tile([P, 1], f32)
                tt(e2f[:, :], S[:, 66:67], rden[:, :], ALU.mult)
                p2 = wk.tile([P, 1], f32)
                tt(p2[:, :], e2i_f, e2f[:, :], ALU.mult)
                rp2 = wk.tile([P, 1], f32)
                nc.vector.reciprocal(rp2[:, :], p2[:, :])

                if STAGE < 36:
                    continue
                # ---- G: expert outputs [P, 15] ----
                outs = wk.tile([P, 15], f32)
                nc.vector.tensor_copy(outs[:, 0:1], imm[:, :])
                tt(outs[:, 1:2], bpf[:, :], imm[:, :], ALU.add)
                ts(outs[:, 2:3], a, axf[:, 0:1], None, ALU.add)
                ts(outs[:, 3:4], a, axf[:, 0:1], None, ALU.subtract)
                ts(outs[:, 4:5], a, axf[:, 0:1], None, ALU.mult)

                # safeb = |ax| < 1e-6 ? 1e-6 : ax
                ab2 = wk.tile([P, 1], f32)
                stt(ab2[:, :], axf[:, :], -1.0, axf[:, :], ALU.mult, ALU.max)
                sm = wk.tile([P, 1], f32)
                ts(sm[:, :], ab2[:, :], 1e-6, None, ALU.is_lt)
                d3 = wk.tile([P, 1], f32)
                ts(d3[:, :], axf[:, :], -1.0, 1e-6, ALU.mult, ALU.add)
                m4 = wk.tile([P, 1], f32)
                tt(m4[:, :], sm[:, :], d3[:, :], ALU.mult)
                safeb = wk.tile([P, 1], f32)
                tt(safeb[:, :], axf[:, :], m4[:, :], ALU.add)
                rb = wk.tile([P, 1], f32)
                nc.vector.reciprocal(rb[:, :], safeb[:, :])
                tt(outs[:, 5:6], a, rb[:, :], ALU.mult)          # div
                fd = floor_big(outs[:, 5:6], "fd")
                m2 = wk.tile([P, 1], f32)
                tt(m2[:, :], safeb[:, :], fd[:, :], ALU.mult)
                tt(outs[:, 6:7], a, m2[:, :], ALU.subtract)      # mod
                tt(outs[:, 7:8], a, p2[:, :], ALU.mult)          # shl
                tt(outs[:, 8:9], a, rp2[:, :], ALU.mult)         # shr
                nc.vector.tensor_copy(outs[:, 9:10], eq2[:, :])
                ts(outs[:, 10:11], eq2[:, :], -1.0, 1.0, ALU.mult, ALU.add)
                nc.vector.tensor_copy(outs[:, 11:12], S[:, 64:65])          # lt
                nc.vector.tensor_copy(outs[:, 12:13], S[:, 65:66])          # gt
                ts(outs[:, 13:14], S[:, 65:66], -1.0, 1.0, ALU.mult, ALU.add)
                ts(outs[:, 14:15], S[:, 64:65], -1.0, 1.0, ALU.mult, ALU.add)

                if STAGE < 37:
                    continue
                # ---- H: combine ----
                prod = wk.tile([P, 15], f32)
                tt(prod[:, :], gates[:, :], outs[:, :], ALU.mult)
                ssum = wk.tile([P, 1], f32)
                nc.vector.tensor_reduce(ssum[:, :], prod[:, :], axis=AXL.X,
                                        op=ALU.add)
                onemg = wk.tile([P, 1], f32)
                ts(onemg[:, :], gsum[:, :], -1.0, 1.0, ALU.mult, ALU.add)
                t4 = wk.tile([P, 1], f32)
                tt(t4[:, :], onemg[:, :], axf[:, :], ALU.mult)
                new_ax = st.tile([P, 1], f32, tag=f"ax{k}")
                tt(new_ax[:, :], ssum[:, :], t4[:, :], ALU.add)

                new_sp = st.tile([P, 1], f32, tag=f"sp{k}")
                stt(new_sp[:, :], popg[:, :], 8.0, spf[:, :], ALU.mult, ALU.add)
                new_pc = st.tile([P, 1], f32, tag=f"pc{k}")
                ts(new_pc[:, :], pcf[:, :], 8.0, None, ALU.add)

                # scatter value: a + popg*(new_ax - a)
                d5 = wk.tile([P, 1], f32)
                tt(d5[:, :], new_ax[:, :], a, ALU.subtract)
                m5 = wk.tile([P, 1], f32)
                tt(m5[:, :], popg[:, :], d5[:, :], ALU.mult)
                val = wk.tile([P, 1], f32)
                tt(val[:, :], a, m5[:, :], ALU.add)

                if STAGE < 38:
                    pcf, spf, axf = new_pc, new_sp, new_ax
                    continue
                # ---- I: scatter into the output + log append ----
                nc.gpsimd.indirect_dma_start(
                    out=out_flat, out_offset=IOff(ap=offm[:, 1:2], axis=0),
                    in_=val[:, :], in_offset=None,
                    bounds_check=P * MEM - 1, oob_is_err=False)

                # log_idx[k] = idx_sp if idx_sp < MEM else -1 ; log_val[k] = val
                m6 = wk.tile([P, 1], f32)
                tt(m6[:, :], gem[:, 1:2], idxf[:, 1:2], ALU.mult)
                t7 = wk.tile([P, 1], f32)
                tt(t7[:, :], idxf[:, 1:2], m6[:, :], ALU.subtract)
                tt(log_idx[:, k:k + 1], t7[:, :], gem[:, 1:2], ALU.subtract)
                nc.vector.tensor_copy(log_val[:, k:k + 1], val[:, :])

                pcf, spf, axf = new_pc, new_sp, new_ax

    if not nc.is_finalized():
        nc.finalize()
    return nc


def _shard_inputs(memory, output, ax, pc, sp, bp, out_len):
    rows = memory.shape[0]
    per = rows // N_CORES
    in_maps = []
    for i in range(N_CORES):
        s = slice(i * per, (i + 1) * per)
        in_maps.append({
            "memory": np.ascontiguousarray(memory[s]).astype(np.float32),
            "output": np.ascontiguousarray(output[s]).astype(np.float32),
            "ax": np.ascontiguousarray(ax[s]).reshape(per, 1).astype(np.float32),
            "pc": np.ascontiguousarray(pc[s]).reshape(per, 1).astype(np.int32),
            "sp": np.ascontiguousarray(sp[s]).reshape(per, 1).astype(np.int32),
            "bp": np.ascontiguousarray(bp[s]).reshape(per, 1).astype(np.int32),
            "out_len": np.ascontiguousarray(out_len[s]).reshape(per, 1).astype(np.int32),
        })
    return in_maps


def kernel(memory, output, ax, pc, sp, bp, out_len, num_steps):
    from concourse import bass_utils

    n = int(num_steps)
    if n not in _BUILD_CACHE:
        _BUILD_CACHE[n] = build_bass(n)
    nc = _BUILD_CACHE[n]
    in_maps = _shard_inputs(np.asarray(memory), np.asarray(output),
                            np.asarray(ax), np.asarray(pc), np.asarray(sp),
                            np.asarray(bp), np.asarray(out_len))
    res = bass_utils.run_bass_kernel_spmd(nc, in_maps,
                                          core_ids=list(range(N_CORES)))
    out = np.concatenate([r["out"] for r in res.results], axis=0)
    return out.astype(np.float32)
